# revision 10
# baseline (speedup 1.0000x reference)
"""Trainium2 Bass kernel for nn_DWNBlock (LRU scan + Lipschitz sandwich MLP).

v3: tri-matmul chunked scan (PE cumsum) + packed rotation products +
5-matmul y fold + chunk-paired fp8-DR MLP.

Per core (batch-parallel, x^T channel-major [128, 8192]):
  1. Bu in [s, n] layout via PE (f32r): psum [128,1024], s-tile i at
     cols 256i..256i+256 = [re n | im n]
  2. prescale by lam^-s (bf16 packed tables, DVE/Pool), shared
     upper-tri-ones matmuls (PE, bf16) -> m[n, t] psum (cumsum +
     transpose in one), carry folded as per-partition bias in the
     psum->bf16 copy (ACT), tiny f32 carry chain ops (DVE)
  3. postscale: packed [pos_re|pos_im] x [m|m] stride-0 products;
     y = C_re@H_re - C_im@H_im + D@x as 5 accumulated matmuls on the
     products directly (H never materialized)
  4. folded MLP: relu(G1 y) bf16 -> fp8 -> G2/G3/G4 fp8 DoubleRow,
     chunk-PAIRED so each DR weight load covers 2 matmuls; residual +x
     folded into the G4 psum group via an identity matmul
Precision: same scan math as the proven baseline (~1.3e-2 vs 2e-2 gate).
"""
import math
import os
import sys

for _p in ('/opt/trn_rl_repo',):
    if _p not in sys.path:
        sys.path.insert(0, _p)

import numpy as np
import ml_dtypes

D = 128          # d_model
N = 128          # d_state
H = 512          # MLP hidden
T = 8192         # sequence length
B = 8            # batch
L = 512          # scan chunk length
NCORES = 8
SCALE = 1.0
SQRT2 = math.sqrt(2.0)

F8NP = ml_dtypes.float8_e4m3
BFNP = ml_dtypes.bfloat16


# ---------------------------------------------------------------- host prep

def _cayley64(W):
    cout, cin = W.shape
    if cin > cout:
        return _cayley64(W.T).T
    U, V = W[:cin], W[cin:]
    I = np.eye(cin, dtype=W.dtype)
    A = U - U.T + V.T @ V
    iIpA = np.linalg.inv(I + A)
    return np.concatenate([iIpA @ (I - A), -2.0 * V @ iIpA], axis=0)


def _host_prep(p):
    """Fold all parameters into device constants."""
    f8 = np.float64
    nu_log = p['nu_log'].astype(f8)
    theta_log = p['theta_log'].astype(f8)
    gamma_log = p['gamma_log'].astype(f8)
    lam = np.exp(-np.exp(nu_log)) * np.exp(1j * np.exp(theta_log))   # [N]
    Beff = np.exp(gamma_log)[:, None] * (p['B_re'].astype(f8) + 1j * p['B_im'].astype(f8))
    beff_w = np.concatenate([Beff.real.T, Beff.imag.T], axis=1)      # [D, 2N]

    s = np.arange(L)
    loglam = np.log(lam)
    pneg = np.exp(-s[:, None] * loglam[None, :])    # [L, N] = lam^-s
    ppos = np.exp(s[None, :] * loglam[:, None])     # [N, L] = lam^t
    lamL = lam ** L
    lamL1 = lam ** (L - 1)

    # Bu psum layout [128, 1024]: s-tile i at [256i:256(i+1)] = [re n | im n],
    # partition = local time p (global s = 128 i + p).
    def _pk(i):   # s-tile i of pneg, [128, N]
        return pneg[i * 128:(i + 1) * 128, :]
    pnegA = np.concatenate(sum([[_pk(i).real, _pk(i).imag] for i in range(4)], []), axis=1)
    pnegB = np.concatenate(sum([[_pk(i).imag, _pk(i).real] for i in range(4)], []), axis=1)
    pneg_pack = np.concatenate([pnegA, pnegB], axis=1)               # [128, 2048]

    ppos2 = np.concatenate([ppos.real, ppos.imag], axis=1)           # [128, 1024]

    # tri_ones[s, u] = 1 if s <= u, over [128, 512]
    tri_ones = (np.arange(128)[:, None] <= np.arange(512)[None, :]).astype(f8)

    # carry-chain per-partition scalar columns
    lamcols = np.stack([
        lam.real, lam.imag, -lam.imag,
        lamL.real, lamL.imag, -lamL.imag,
        lamL1.real, lamL1.imag, -lamL1.imag,
    ], axis=1)                                       # [128, 9]

    C = p['C_re'].astype(f8) + 1j * p['C_im'].astype(f8)             # [D, N]
    # postA = [pos_re*m_re | pos_im*m_re] = [P1|P3]
    # postB = [pos_re*m_im | pos_im*m_im] = [P4|P2]
    # y = C_re@P1 - C_im@P3 - C_im@P4 - C_re@P2
    ytw4 = np.concatenate([C.real.T, -C.imag.T, -C.imag.T, -C.real.T], axis=1)  # [N, 4D]
    ytw_x = np.ascontiguousarray(p['Dmat'].astype(f8).T)             # [128, 128]
    ident = np.eye(128, dtype=f8)

    def _q(Wkey, akey, fout):
        Wd = p[Wkey].astype(f8)
        Q = _cayley64((float(p[akey][0]) / np.linalg.norm(Wd)) * Wd)
        return Q[:, fout:], Q[:, :fout]

    Q1in, Q1out = _q('W1', 'alpha1', H)
    Q2in, Q2out = _q('W2', 'alpha2', H)
    Q3in, Q3out = _q('W3', 'alpha3', H)
    Qlin = _cayley64((float(p['alphal'][0]) / np.linalg.norm(p['Wl'].astype(f8)))
                     * p['Wl'].astype(f8))[:, D:]    # [128, 512]

    e = np.exp
    ps1, ps2, ps3 = p['psi1'].astype(f8), p['psi2'].astype(f8), p['psi3'].astype(f8)
    G1 = SCALE * SCALE * SQRT2 * (Q1in.T * e(-ps1)[None, :])                    # [128, 512]
    G2 = 2.0 * SCALE * (e(ps1)[:, None] * Q1out) @ (Q2in.T * e(-ps2)[None, :])  # [512, 512]
    G3 = 2.0 * SCALE * (e(ps2)[:, None] * Q2out) @ (Q3in.T * e(-ps3)[None, :])  # [512, 512]
    G4 = SQRT2 * SCALE * (e(ps3)[:, None] * Q3out) @ Qlin.T                     # [512, 128]

    def pack_kt(G, width):     # [K, width] -> [128, K//128 * width] k-tile-major
        K = G.shape[0]
        return np.concatenate([G[k * 128:(k + 1) * 128, :] for k in range(K // 128)], axis=1)

    out32 = dict(beff_w=beff_w, lamcols=lamcols, ytw_x=ytw_x, ident=ident)
    outbf = dict(pneg_pack=pneg_pack, ppos2=ppos2, tri_ones=tri_ones,
                 ytw4=ytw4, g1=G1)
    outf8 = dict(g2=pack_kt(G2, 512), g3=pack_kt(G3, 512), g4=pack_kt(G4, 128))
    res = {k: np.ascontiguousarray(v, dtype=np.float32) for k, v in out32.items()}
    res.update({k: np.ascontiguousarray(v.astype(np.float32), dtype=BFNP)
                for k, v in outbf.items()})
    res.update({k: np.ascontiguousarray(v.astype(np.float32), dtype=F8NP)
                for k, v in outf8.items()})
    return res


# ---------------------------------------------------------------- device program

def _build_program(t_len, reps=1):
    from contextlib import nullcontext
    from concourse import bacc
    import concourse.mybir as mybir
    from concourse.tile import TileContext

    f32 = mybir.dt.float32
    f32r = mybir.dt.float32r
    bf16 = mybir.dt.bfloat16
    fp8 = mybir.dt.float8e4
    nchunk = t_len // L

    nc = bacc.Bacc("TRN2", target_bir_lowering=False, debug=False)

    xt_d = nc.dram_tensor("xt", [128, t_len], f32r, kind="ExternalInput").ap()
    beff_d = nc.dram_tensor("beff_w", [128, 256], f32r, kind="ExternalInput").ap()
    pneg_d = nc.dram_tensor("pneg_pack", [128, 2048], bf16, kind="ExternalInput").ap()
    ppos2_d = nc.dram_tensor("ppos2", [128, 1024], bf16, kind="ExternalInput").ap()
    tri_d = nc.dram_tensor("tri_ones", [128, 512], bf16, kind="ExternalInput").ap()
    lamc_d = nc.dram_tensor("lamcols", [128, 9], f32, kind="ExternalInput").ap()
    ytw4_d = nc.dram_tensor("ytw4", [128, 512], bf16, kind="ExternalInput").ap()
    ytwx_d = nc.dram_tensor("ytw_x", [128, 128], f32r, kind="ExternalInput").ap()
    ident_d = nc.dram_tensor("ident", [128, 128], f32r, kind="ExternalInput").ap()
    g1_d = nc.dram_tensor("g1", [128, 512], bf16, kind="ExternalInput").ap()
    g2_d = nc.dram_tensor("g2", [128, 2048], fp8, kind="ExternalInput").ap()
    g3_d = nc.dram_tensor("g3", [128, 2048], fp8, kind="ExternalInput").ap()
    g4_d = nc.dram_tensor("g4", [128, 512], fp8, kind="ExternalInput").ap()
    out_d = nc.dram_tensor("outT", [128, t_len], f32, kind="ExternalOutput").ap()

    env = dict(nchunk=nchunk, out_d=out_d)

    with TileContext(nc) as tc:
        with (
            tc.tile_pool(name="const", bufs=1) as cpool,
            tc.tile_pool(name="bpool", bufs=3) as bpool,       # busb bf16
            tc.tile_pool(name="epool", bufs=3) as epool,       # e1/e2
            tc.tile_pool(name="postpool", bufs=3) as postpool, # postA/postB
            tc.tile_pool(name="ypool", bufs=4) as ypool,       # y_sb bf16
            tc.tile_pool(name="zw", bufs=2) as zw_pool,        # fp8 activations
            tc.tile_pool(name="opool", bufs=3) as opool,       # out f32
            tc.tile_pool(name="carry", bufs=4) as carry_pool,
            tc.tile_pool(name="bups", bufs=1, space="PSUM") as bu_ps,
            tc.tile_pool(name="myps", bufs=1, space="PSUM") as my_ps,
            tc.tile_pool(name="zps", bufs=2, space="PSUM") as z_ps,
        ):
            # ---- constants into SBUF
            xt = cpool.tile([128, t_len], f32r, tag="xt")
            for q in range(max(1, t_len // 2048)):
                w = min(2048, t_len)
                nc.sync.dma_start(xt[:, q * w:(q + 1) * w], xt_d[:, q * w:(q + 1) * w])
            beff = cpool.tile([128, 256], f32r, tag="beff")
            nc.sync.dma_start(beff[:], beff_d[:])
            pneg = cpool.tile([128, 2048], bf16, tag="pneg")
            nc.sync.dma_start(pneg[:], pneg_d[:])
            ppos2 = cpool.tile([128, 1024], bf16, tag="ppos2")
            nc.sync.dma_start(ppos2[:], ppos2_d[:])
            tri = cpool.tile([128, 512], bf16, tag="tri")
            nc.sync.dma_start(tri[:], tri_d[:])
            lamc = cpool.tile([128, 9], f32, tag="lamc")
            nc.sync.dma_start(lamc[:], lamc_d[:])
            ytw4 = cpool.tile([128, 512], bf16, tag="ytw4")
            nc.sync.dma_start(ytw4[:], ytw4_d[:])
            ytw_x = cpool.tile([128, 128], f32r, tag="ytwx")
            nc.sync.dma_start(ytw_x[:], ytwx_d[:])
            ident = cpool.tile([128, 128], f32r, tag="ident")
            nc.sync.dma_start(ident[:], ident_d[:])
            g1 = cpool.tile([128, 512], bf16, tag="g1")
            nc.sync.dma_start(g1[:], g1_d[:])
            g2 = cpool.tile([128, 4, 512], fp8, tag="g2")
            nc.sync.dma_start(g2[:], g2_d[:])
            g3 = cpool.tile([128, 4, 512], fp8, tag="g3")
            nc.sync.dma_start(g3[:], g3_d[:])
            g4 = cpool.tile([128, 4, 128], fp8, tag="g4")
            nc.sync.dma_start(g4[:], g4_d[:])
            bias0 = cpool.tile([128, 1], f32, tag="bias0")
            nc.vector.memset(bias0[:], 0.0)

            env.update(xt=xt, beff=beff, pneg=pneg, ppos2=ppos2, tri=tri,
                       lamc=lamc, ytw4=ytw4, ytw_x=ytw_x, ident=ident,
                       g1=g1, g2=g2, g3=g3, g4=g4, bias0=bias0,
                       bpool=bpool, epool=epool, postpool=postpool,
                       ypool=ypool, zw_pool=zw_pool, opool=opool,
                       carry_pool=carry_pool,
                       bu_ps=bu_ps, my_ps=my_ps, z_ps=z_ps)

            loop_cm = tc.For_i(0, reps) if reps > 1 else nullcontext()
            with loop_cm:
                _body(nc, tc, env)

    nc.finalize()
    return nc


def _body(nc, tc, env):
    import concourse.mybir as mybir
    from collections import deque
    f32 = mybir.dt.float32
    f32r = mybir.dt.float32r
    bf16 = mybir.dt.bfloat16
    fp8 = mybir.dt.float8e4
    AL = mybir.AluOpType
    ACT = mybir.ActivationFunctionType
    DR = mybir.MatmulPerfMode.DoubleRow

    (nchunk, out_d, xt, beff, pneg, ppos2, tri, lamc, ytw4, ytw_x, ident,
     g1, g2, g3, g4, bias0, bpool, epool, postpool, ypool, zw_pool, opool,
     carry_pool, bu_ps, my_ps, z_ps) = (
        env['nchunk'], env['out_d'], env['xt'], env['beff'], env['pneg'],
        env['ppos2'], env['tri'], env['lamc'], env['ytw4'], env['ytw_x'],
        env['ident'], env['g1'], env['g2'], env['g3'], env['g4'],
        env['bias0'], env['bpool'], env['epool'], env['postpool'],
        env['ypool'], env['zw_pool'], env['opool'], env['carry_pool'],
        env['bu_ps'], env['my_ps'], env['z_ps'])

    lam_re, lam_im, lam_imn = lamc[:, 0:1], lamc[:, 1:2], lamc[:, 2:3]
    lamL_re, lamL_im, lamL_imn = lamc[:, 3:4], lamc[:, 4:5], lamc[:, 5:6]
    lamL1_re, lamL1_im, lamL1_imn = lamc[:, 6:7], lamc[:, 7:8], lamc[:, 8:9]
    ppos2v = ppos2[:].rearrange("p (i n) -> p i n", i=2)

    state = dict(h_re=None, h_im=None, mp_re=None, mp_im=None)
    BU, BUSB, UP, MSB, PA, PB, YSB = {}, {}, {}, {}, {}, {}, {}
    Z1, Z2, Z3 = {}, {}, {}

    relu_rr = [0]
    # alternate ACT/DVE; ~half each
    def relu(dst, src_ps):
        if relu_rr[0] % 2 == 0:
            nc.scalar.activation(dst, src_ps, ACT.Relu, bias=bias0[:])
        else:
            nc.vector.tensor_scalar_max(dst, src_ps, 0.0)
        relu_rr[0] += 1

    # ---- stage S0: Bu matmuls (PE)
    def s_bu(c):
        t0 = c * L
        bu = bu_ps.tile([128, 1024], f32, tag="bu")
        for i in range(4):
            lhs = xt[:, t0 + i * 128: t0 + (i + 1) * 128]
            nc.tensor.matmul(bu[:, i * 256:(i + 1) * 256], lhs, beff[:],
                             start=True, stop=True)
        BU[c] = bu

    # ---- stage S1: psum->bf16, prescale products + folds
    def s_pre(c):
        bu = BU.pop(c)
        busb = bpool.tile([128, 1024], bf16, tag="busb")
        nc.scalar.activation(busb[:], bu[:], ACT.Identity, bias=bias0[:])
        e1 = epool.tile([128, 1024], bf16, tag="e1")
        e2 = epool.tile([128, 1024], bf16, tag="e2")
        nc.gpsimd.tensor_tensor(e1[:], busb[:], pneg[:, 0:1024], AL.mult)
        nc.gpsimd.tensor_tensor(e2[:], busb[:], pneg[:, 1024:2048], AL.mult)
        e1v = e1[:].rearrange("p (i h n) -> p i h n", i=4, h=2)
        e2v = e2[:].rearrange("p (i h n) -> p i h n", i=4, h=2)
        up = epool.tile([128, 4, 256], bf16, tag="up")
        upv = up[:].rearrange("p i (h n) -> p i h n", h=2)
        nc.vector.tensor_tensor(upv[:, :, 0, :], e1v[:, :, 0, :], e1v[:, :, 1, :],
                                AL.subtract)
        nc.vector.tensor_tensor(upv[:, :, 1, :], e2v[:, :, 0, :], e2v[:, :, 1, :],
                                AL.add)
        UP[c] = up

    # ---- stage S2: carry ops, tri matmuls, mp, msb
    def s_tri(c):
        up = UP.pop(c)
        if c > 0:
            mp_re, mp_im = state['mp_re'], state['mp_im']
            h_re, h_im = state['h_re'], state['h_im']
            c1 = carry_pool.tile([128, 1], f32, tag="c1")
            h_re_n = carry_pool.tile([128, 1], f32, tag="hre")
            d1 = carry_pool.tile([128, 1], f32, tag="d1")
            h_im_n = carry_pool.tile([128, 1], f32, tag="him")
            nc.vector.scalar_tensor_tensor(c1[:], h_re[:], lamL_re, mp_re[:], AL.mult, AL.add)
            nc.vector.scalar_tensor_tensor(h_re_n[:], h_im[:], lamL_imn, c1[:], AL.mult, AL.add)
            nc.vector.scalar_tensor_tensor(d1[:], h_im[:], lamL_re, mp_im[:], AL.mult, AL.add)
            nc.vector.scalar_tensor_tensor(h_im_n[:], h_re[:], lamL_im, d1[:], AL.mult, AL.add)
            state['h_re'], state['h_im'] = h_re_n, h_im_n
            h_re, h_im = h_re_n, h_im_n
            a_re = carry_pool.tile([128, 1], f32, tag="are")
            a_im = carry_pool.tile([128, 1], f32, tag="aim")
            t1 = carry_pool.tile([128, 1], f32, tag="ct1")
            t2 = carry_pool.tile([128, 1], f32, tag="ct2")
            nc.vector.tensor_tensor(t1[:], h_im[:], lam_imn, AL.mult)
            nc.vector.scalar_tensor_tensor(a_re[:], h_re[:], lam_re, t1[:], AL.mult, AL.add)
            nc.vector.tensor_tensor(t2[:], h_re[:], lam_im, AL.mult)
            nc.vector.scalar_tensor_tensor(a_im[:], h_im[:], lam_re, t2[:], AL.mult, AL.add)
        else:
            a_re = carry_pool.tile([128, 1], f32, tag="are")
            a_im = carry_pool.tile([128, 1], f32, tag="aim")
            nc.vector.memset(a_re[:], 0.0)
            nc.vector.memset(a_im[:], 0.0)
            h0_re = carry_pool.tile([128, 1], f32, tag="hre")
            h0_im = carry_pool.tile([128, 1], f32, tag="him")
            nc.vector.memset(h0_re[:], 0.0)
            nc.vector.memset(h0_im[:], 0.0)
            state['h_re'], state['h_im'] = h0_re, h0_im
        m = my_ps.tile([128, 1024], f32, tag="my")
        for j in range(4):
            width = 512 - 128 * j
            nc.tensor.matmul(m[:, 128 * j:512], up[:, j, 0:128], tri[:, 0:width],
                             start=(j == 0), stop=(j == 3))
        for j in range(4):
            width = 512 - 128 * j
            nc.tensor.matmul(m[:, 512 + 128 * j:1024], up[:, j, 128:256],
                             tri[:, 0:width], start=(j == 0), stop=(j == 3))
        mr_col = m[:, 511:512]
        mi_col = m[:, 1023:1024]
        mp1 = carry_pool.tile([128, 1], f32, tag="mp1")
        mp_re = carry_pool.tile([128, 1], f32, tag="mpre")
        mp2 = carry_pool.tile([128, 1], f32, tag="mp2")
        mp_im = carry_pool.tile([128, 1], f32, tag="mpim")
        nc.vector.tensor_tensor(mp1[:], mr_col, lamL1_re, AL.mult)
        nc.vector.scalar_tensor_tensor(mp_re[:], mi_col, lamL1_imn, mp1[:], AL.mult, AL.add)
        nc.vector.tensor_tensor(mp2[:], mi_col, lamL1_re, AL.mult)
        nc.vector.scalar_tensor_tensor(mp_im[:], mr_col, lamL1_im, mp2[:], AL.mult, AL.add)
        state['mp_re'], state['mp_im'] = mp_re, mp_im
        msb = bpool.tile([128, 1024], bf16, tag="msb")
        nc.scalar.activation(msb[:, 0:512], m[:, 0:512], ACT.Identity, bias=a_re[:])
        nc.scalar.activation(msb[:, 512:1024], m[:, 512:1024], ACT.Identity, bias=a_im[:])
        MSB[c] = msb

    # ---- stage S3: postscale products, y matmuls, y_sb
    def s_post(c):
        t0 = c * L
        msb = MSB.pop(c)
        msb_re2 = msb[:, 0:512].unsqueeze(1).broadcast_to([128, 2, 512])
        msb_im2 = msb[:, 512:1024].unsqueeze(1).broadcast_to([128, 2, 512])
        postA = postpool.tile([128, 2, 512], bf16, tag="postA")
        postB = postpool.tile([128, 2, 512], bf16, tag="postB")
        nc.vector.tensor_tensor(postA[:], msb_re2, ppos2v, AL.mult)
        nc.gpsimd.tensor_tensor(postB[:], msb_im2, ppos2v, AL.mult)
        yp = z_ps.tile([128, 1024], f32, tag="zps", name="yp")
        y = yp[:, 0:512]
        nc.tensor.matmul(y, ytw4[:, 0:128], postA[:, 0, :], start=True, stop=False)
        nc.tensor.matmul(y, ytw4[:, 128:256], postA[:, 1, :], start=False, stop=False)
        nc.tensor.matmul(y, ytw4[:, 256:384], postB[:, 0, :], start=False, stop=False)
        nc.tensor.matmul(y, ytw4[:, 384:512], postB[:, 1, :], start=False, stop=False)
        nc.tensor.matmul(y, ytw_x[:], xt[:, t0:t0 + 512], start=False, stop=True)
        # y_sb written into the pair tile half (pair = c//2)
        pair = c // 2
        if pair not in YSB:
            YSB[pair] = ypool.tile([128, 1024], bf16, tag="ysb", name="ysb")
        nc.scalar.activation(YSB[pair][:, (c % 2) * 512:(c % 2) * 512 + 512], y,
                             ACT.Identity, bias=bias0[:])

    # ---- MLP stages (per quad q4: chunks 4q4 .. 4q4+3, pairs 2q4, 2q4+1)
    def m_g1(q4, which):      # which: 0 = pairA, 1 = pairB
        pair = 2 * q4 + which
        ysb = YSB.pop(pair)
        if q4 not in Z1:
            Z1[q4] = zw_pool.tile([128, 4, 2048], fp8, tag="z1w", name="z1w")
        z1w = Z1[q4]
        base = which * 1024
        for mi in range(4):
            zp = z_ps.tile([128, 1024], f32, tag="zps")
            gsl = g1[:, mi * 128:(mi + 1) * 128]
            nc.tensor.matmul(zp[:, 0:512], gsl, ysb[:, 0:512], start=True, stop=True)
            nc.tensor.matmul(zp[:, 512:1024], gsl, ysb[:, 512:1024], start=True, stop=True)
            relu(z1w[:, mi, base:base + 1024], zp[:])

    def m_dr(q4, gw, zin_d, zout_d):
        zin = zin_d.pop(q4)
        zout = zw_pool.tile([128, 4, 2048], fp8,
                            tag="z2w" if zout_d is Z2 else "z3w")
        zout_d[q4] = zout
        for mi in range(4):
            zpA = z_ps.tile([128, 1024], f32, tag="zps")
            zpB = z_ps.tile([128, 1024], f32, tag="zps")
            for q in range(2):
                gsl = gw[:, 2 * q:2 * q + 2, mi * 128:(mi + 1) * 128]
                for zp, base in ((zpA, 0), (zpB, 1024)):
                    for hh in range(2):
                        nc.tensor.matmul(
                            zp[:, hh * 512:(hh + 1) * 512],
                            gsl,
                            zin[:, 2 * q:2 * q + 2, base + hh * 512:base + (hh + 1) * 512],
                            start=(q == 0), stop=(q == 1),
                            perf_mode=DR, skip_group_check=True)
            relu(zout[:, mi, 0:1024], zpA[:])
            relu(zout[:, mi, 1024:2048], zpB[:])

    def m_g4(q4):
        z3w = Z3.pop(q4)
        t0 = 4 * q4 * L
        zpA = z_ps.tile([128, 1024], f32, tag="zps")
        zpB = z_ps.tile([128, 1024], f32, tag="zps")
        halves = [(zpA, 0, t0), (zpA, 1, t0 + L), (zpB, 0, t0 + 2 * L), (zpB, 1, t0 + 3 * L)]
        for q in range(2):
            gsl = g4[:, 2 * q:2 * q + 2, :]
            for i, (zp, hh, tx) in enumerate(halves):
                nc.tensor.matmul(zp[:, hh * 512:(hh + 1) * 512], gsl,
                                 z3w[:, 2 * q:2 * q + 2, i * 512:(i + 1) * 512],
                                 start=(q == 0), stop=False,
                                 perf_mode=DR, skip_group_check=True)
        for zp, hh, tx in halves:
            nc.tensor.matmul(zp[:, hh * 512:(hh + 1) * 512], ident[:],
                             xt[:, tx:tx + 512], start=False, stop=True,
                             skip_group_check=True)
        for zp, name in ((zpA, "oA"), (zpB, "oB")):
            o_sb = opool.tile([128, 1024], f32, tag="osb")
            nc.scalar.activation(o_sb[:], zp[:], ACT.Identity, bias=bias0[:])
            tx = t0 if name == "oA" else t0 + 2 * L
            nc.sync.dma_start(out_d[:, tx:tx + 512], o_sb[:, 0:512])
            nc.sync.dma_start(out_d[:, tx + 512:tx + 1024], o_sb[:, 512:1024])

    # ---- modulo-scheduled wave
    mlp_q = deque()
    for t in range(nchunk + 4):
        c_post = t - 3
        c_tri = t - 2
        c_pre = t - 1
        if 0 <= c_post < nchunk:
            s_post(c_post)
            if c_post % 4 == 3:
                q4 = c_post // 4
                mlp_q.append(lambda q4=q4: m_g1(q4, 0))
                mlp_q.append(lambda q4=q4: m_g1(q4, 1))
                mlp_q.append(lambda q4=q4: m_dr(q4, g2, Z1, Z2))
                mlp_q.append(lambda q4=q4: m_dr(q4, g3, Z2, Z3))
                mlp_q.append(lambda q4=q4: m_g4(q4))
        if 0 <= c_tri < nchunk:
            s_tri(c_tri)
        if 0 <= c_pre < nchunk:
            s_pre(c_pre)
        if t < nchunk:
            s_bu(t)
        if mlp_q:
            mlp_q.popleft()()
    while mlp_q:
        mlp_q.popleft()()


# ---------------------------------------------------------------- PJRT runner

def _make_runner(nc, n_cores):
    import jax
    from jax.sharding import Mesh, PartitionSpec
    from jax.experimental.shard_map import shard_map
    import concourse.mybir as mybir
    from concourse import bass2jax

    bass2jax.install_neuronx_cc_hook()
    assert nc.is_finalized()
    partition_name = nc.partition_id_tensor.name if nc.partition_id_tensor else None

    in_names, out_names, out_avals, zero_shapes = [], [], [], []
    for alloc in nc.m.functions[0].allocations:
        if not isinstance(alloc, mybir.MemoryLocationSet):
            continue
        name = alloc.memorylocations[0].name
        if alloc.kind == "ExternalInput":
            if name != partition_name:
                in_names.append(name)
        elif alloc.kind == "ExternalOutput":
            shape = tuple(alloc.tensor_shape)
            dtype = mybir.dt.np(alloc.dtype)
            out_names.append(name)
            out_avals.append(jax.core.ShapedArray(shape, dtype))
            zero_shapes.append((shape, dtype))
    n_params = len(in_names)
    n_outs = len(out_avals)
    all_in_names = list(in_names) + list(out_names)
    if partition_name is not None:
        all_in_names.append(partition_name)
    donate = tuple(range(n_params, n_params + n_outs))

    def _body_fn(*args):
        operands = list(args)
        if partition_name is not None:
            operands.append(bass2jax.partition_id_tensor())
        outs = bass2jax._bass_exec_p.bind(
            *operands,
            out_avals=tuple(out_avals),
            in_names=tuple(all_in_names),
            out_names=tuple(out_names),
            lowering_input_output_aliases=(),
            sim_require_finite=True,
            sim_require_nnan=True,
            nc=nc,
        )
        return tuple(outs)

    devices = jax.devices()[:n_cores]
    if n_cores == 1:
        fn = jax.jit(_body_fn, donate_argnums=donate, keep_unused=True)
    else:
        mesh = Mesh(np.asarray(devices), ("core",))
        fn = jax.jit(
            shard_map(_body_fn, mesh=mesh,
                      in_specs=(PartitionSpec("core"),) * (n_params + n_outs),
                      out_specs=(PartitionSpec("core"),) * n_outs,
                      check_rep=False),
            donate_argnums=donate, keep_unused=True,
        )

    def run(per_core_inputs):
        if n_cores == 1:
            ins = [np.asarray(per_core_inputs[0][n]) for n in in_names]
            zeros = [np.zeros(s, d) for s, d in zero_shapes]
        else:
            ins = [np.concatenate([np.asarray(per_core_inputs[c][n])
                                   for c in range(n_cores)], axis=0) for n in in_names]
            zeros = [np.zeros((n_cores * s[0], *s[1:]), d) for s, d in zero_shapes]
        out_arrs = fn(*ins, *zeros)
        if n_cores == 1:
            return [{name: np.asarray(out_arrs[i]) for i, name in enumerate(out_names)}]
        res = []
        for c in range(n_cores):
            d = {}
            for i, name in enumerate(out_names):
                full = np.asarray(out_arrs[i])
                d[name] = full.reshape(n_cores, *out_avals[i].shape)[c]
            res.append(d)
        return res

    run.fn = fn
    run.in_names = in_names
    run.out_names = out_names
    run.zero_shapes = zero_shapes
    return run


_RUNNER = None


def _get_runner():
    global _RUNNER
    if _RUNNER is None:
        nc = _build_program(T)
        _RUNNER = _make_runner(nc, NCORES)
    return _RUNNER


def kernel(**inputs):
    import time as _time
    global _RUNNER
    p = {k: np.asarray(v) for k, v in inputs.items()}
    consts = _host_prep(p)
    x = p['x'].astype(np.float32)            # [B, T, D]
    per_core = []
    for b in range(B):
        m = dict(consts)
        m['xt'] = np.ascontiguousarray(x[b].T)
        per_core.append(m)
    res = None
    for attempt in range(3):
        try:
            run = _get_runner()
            res = run(per_core)
            break
        except Exception:
            # transient NRT exec faults have been observed on the first
            # execution of a freshly compiled NEFF; rebuild the jitted
            # callable (NEFF comes from the compile cache) and retry.
            _RUNNER = None
            if attempt == 2:
                raise
            _time.sleep(2.0)
    out = np.stack([res[b]['outT'].T for b in range(B)], axis=0)
    return np.ascontiguousarray(out, dtype=np.float32)


# revision 13
# speedup vs baseline: 1.2019x; 1.2019x over previous
"""Trainium2 Bass kernel for nn_DWNBlock (LRU scan + Lipschitz sandwich MLP).

v3: tri-matmul chunked scan (PE cumsum) + packed rotation products +
5-matmul y fold + chunk-paired fp8-DR MLP.

Per core (batch-parallel, x^T channel-major [128, 8192]):
  1. Bu in [s, n] layout via PE (f32r): psum [128,1024], s-tile i at
     cols 256i..256i+256 = [re n | im n]
  2. prescale by lam^-s (bf16 packed tables, DVE/Pool), shared
     upper-tri-ones matmuls (PE, bf16) -> m[n, t] psum (cumsum +
     transpose in one), carry folded as per-partition bias in the
     psum->bf16 copy (ACT), tiny f32 carry chain ops (DVE)
  3. postscale: packed [pos_re|pos_im] x [m|m] stride-0 products;
     y = C_re@H_re - C_im@H_im + D@x as 5 accumulated matmuls on the
     products directly (H never materialized)
  4. folded MLP: relu(G1 y) bf16 -> fp8 -> G2/G3/G4 fp8 DoubleRow,
     chunk-PAIRED so each DR weight load covers 2 matmuls; residual +x
     folded into the G4 psum group via an identity matmul
Precision: same scan math as the proven baseline (~1.3e-2 vs 2e-2 gate).
"""
import math
import os
import sys

for _p in ('/opt/trn_rl_repo',):
    if _p not in sys.path:
        sys.path.insert(0, _p)

import numpy as np
import ml_dtypes

D = 128          # d_model
N = 128          # d_state
H = 512          # MLP hidden
T = 8192         # sequence length
B = 8            # batch
L = 512          # scan chunk length
NCORES = 8
SCALE = 1.0
SQRT2 = math.sqrt(2.0)

F8NP = ml_dtypes.float8_e4m3
BFNP = ml_dtypes.bfloat16


# ---------------------------------------------------------------- host prep

def _cayley64(W):
    cout, cin = W.shape
    if cin > cout:
        return _cayley64(W.T).T
    U, V = W[:cin], W[cin:]
    I = np.eye(cin, dtype=W.dtype)
    A = U - U.T + V.T @ V
    iIpA = np.linalg.inv(I + A)
    return np.concatenate([iIpA @ (I - A), -2.0 * V @ iIpA], axis=0)


def _host_prep(p):
    """Fold all parameters into device constants."""
    f8 = np.float64
    nu_log = p['nu_log'].astype(f8)
    theta_log = p['theta_log'].astype(f8)
    gamma_log = p['gamma_log'].astype(f8)
    lam = np.exp(-np.exp(nu_log)) * np.exp(1j * np.exp(theta_log))   # [N]
    Beff = np.exp(gamma_log)[:, None] * (p['B_re'].astype(f8) + 1j * p['B_im'].astype(f8))
    beff_w = np.concatenate([Beff.real.T, Beff.imag.T], axis=1)      # [D, 2N]

    s = np.arange(L)
    loglam = np.log(lam)
    pneg = np.exp(-s[:, None] * loglam[None, :])    # [L, N] = lam^-s
    ppos = np.exp(s[None, :] * loglam[:, None])     # [N, L] = lam^t
    lamL = lam ** L
    lamL1 = lam ** (L - 1)

    # Bu psum layout [128, 1024]: s-tile i at [256i:256(i+1)] = [re n | im n],
    # partition = local time p (global s = 128 i + p).
    def _pk(i):   # s-tile i of pneg, [128, N]
        return pneg[i * 128:(i + 1) * 128, :]
    pnegA = np.concatenate(sum([[_pk(i).real, _pk(i).imag] for i in range(4)], []), axis=1)
    pnegB = np.concatenate(sum([[_pk(i).imag, _pk(i).real] for i in range(4)], []), axis=1)
    pneg_pack = np.concatenate([pnegA, pnegB], axis=1)               # [128, 2048]

    ppos2 = np.concatenate([ppos.real, ppos.imag], axis=1)           # [128, 1024]

    # tri_ones[s, u] = 1 if s <= u, over [128, 512]
    tri_ones = (np.arange(128)[:, None] <= np.arange(512)[None, :]).astype(f8)

    # carry-chain per-partition scalar columns
    lamcols = np.stack([
        lam.real, lam.imag, -lam.imag,
        lamL.real, lamL.imag, -lamL.imag,
        lamL1.real, lamL1.imag, -lamL1.imag,
    ], axis=1)                                       # [128, 9]

    C = p['C_re'].astype(f8) + 1j * p['C_im'].astype(f8)             # [D, N]
    # postA = [pos_re*m_re | pos_im*m_re] = [P1|P3]
    # postB = [pos_re*m_im | pos_im*m_im] = [P4|P2]
    # y = C_re@P1 - C_im@P3 - C_im@P4 - C_re@P2
    ytw4 = np.concatenate([C.real.T, -C.imag.T, -C.imag.T, -C.real.T], axis=1)  # [N, 4D]
    ytw_x = np.ascontiguousarray(p['Dmat'].astype(f8).T)             # [128, 128]
    ident = np.eye(128, dtype=f8)

    def _q(Wkey, akey, fout):
        Wd = p[Wkey].astype(f8)
        Q = _cayley64((float(p[akey][0]) / np.linalg.norm(Wd)) * Wd)
        return Q[:, fout:], Q[:, :fout]

    Q1in, Q1out = _q('W1', 'alpha1', H)
    Q2in, Q2out = _q('W2', 'alpha2', H)
    Q3in, Q3out = _q('W3', 'alpha3', H)
    Qlin = _cayley64((float(p['alphal'][0]) / np.linalg.norm(p['Wl'].astype(f8)))
                     * p['Wl'].astype(f8))[:, D:]    # [128, 512]

    e = np.exp
    ps1, ps2, ps3 = p['psi1'].astype(f8), p['psi2'].astype(f8), p['psi3'].astype(f8)
    G1 = SCALE * SCALE * SQRT2 * (Q1in.T * e(-ps1)[None, :])                    # [128, 512]
    G2 = 2.0 * SCALE * (e(ps1)[:, None] * Q1out) @ (Q2in.T * e(-ps2)[None, :])  # [512, 512]
    G3 = 2.0 * SCALE * (e(ps2)[:, None] * Q2out) @ (Q3in.T * e(-ps3)[None, :])  # [512, 512]
    G4 = SQRT2 * SCALE * (e(ps3)[:, None] * Q3out) @ Qlin.T                     # [512, 128]

    def pack_kt(G, width):     # [K, width] -> [128, K//128 * width] k-tile-major
        K = G.shape[0]
        return np.concatenate([G[k * 128:(k + 1) * 128, :] for k in range(K // 128)], axis=1)

    out32 = dict(beff_w=beff_w, lamcols=lamcols, ytw_x=ytw_x, ident=ident)
    outbf = dict(pneg_pack=pneg_pack, ppos2=ppos2, tri_ones=tri_ones,
                 ytw4=ytw4, g1=G1)
    outf8 = dict(g2=pack_kt(G2, 512), g3=pack_kt(G3, 512), g4=pack_kt(G4, 128))
    res = {k: np.ascontiguousarray(v, dtype=np.float32) for k, v in out32.items()}
    res.update({k: np.ascontiguousarray(v.astype(np.float32), dtype=BFNP)
                for k, v in outbf.items()})
    res.update({k: np.ascontiguousarray(v.astype(np.float32), dtype=F8NP)
                for k, v in outf8.items()})
    return res


# ---------------------------------------------------------------- device program

def _build_program(t_len, reps=1):
    from contextlib import nullcontext
    from concourse import bacc
    import concourse.mybir as mybir
    from concourse.tile import TileContext

    f32 = mybir.dt.float32
    f32r = mybir.dt.float32r
    bf16 = mybir.dt.bfloat16
    fp8 = mybir.dt.float8e4
    nchunk = t_len // L

    nc = bacc.Bacc("TRN2", target_bir_lowering=False, debug=False)

    xt_d = nc.dram_tensor("xt", [128, t_len], f32r, kind="ExternalInput").ap()
    beff_d = nc.dram_tensor("beff_w", [128, 256], f32r, kind="ExternalInput").ap()
    pneg_d = nc.dram_tensor("pneg_pack", [128, 2048], bf16, kind="ExternalInput").ap()
    ppos2_d = nc.dram_tensor("ppos2", [128, 1024], bf16, kind="ExternalInput").ap()
    tri_d = nc.dram_tensor("tri_ones", [128, 512], bf16, kind="ExternalInput").ap()
    lamc_d = nc.dram_tensor("lamcols", [128, 9], f32, kind="ExternalInput").ap()
    ytw4_d = nc.dram_tensor("ytw4", [128, 512], bf16, kind="ExternalInput").ap()
    ytwx_d = nc.dram_tensor("ytw_x", [128, 128], f32r, kind="ExternalInput").ap()
    ident_d = nc.dram_tensor("ident", [128, 128], f32r, kind="ExternalInput").ap()
    g1_d = nc.dram_tensor("g1", [128, 512], bf16, kind="ExternalInput").ap()
    g2_d = nc.dram_tensor("g2", [128, 2048], fp8, kind="ExternalInput").ap()
    g3_d = nc.dram_tensor("g3", [128, 2048], fp8, kind="ExternalInput").ap()
    g4_d = nc.dram_tensor("g4", [128, 512], fp8, kind="ExternalInput").ap()
    out_d = nc.dram_tensor("outT", [128, t_len], f32, kind="ExternalOutput").ap()

    env = dict(nchunk=nchunk, out_d=out_d)

    with TileContext(nc) as tc:
        with (
            tc.tile_pool(name="const", bufs=1) as cpool,
            tc.tile_pool(name="bpool", bufs=3) as bpool,       # busb bf16
            tc.tile_pool(name="epool", bufs=3) as epool,       # e1/e2
            tc.tile_pool(name="postpool", bufs=3) as postpool, # postA/postB
            tc.tile_pool(name="ypool", bufs=4) as ypool,       # y_sb bf16
            tc.tile_pool(name="zw", bufs=2) as zw_pool,        # fp8 activations
            tc.tile_pool(name="opool", bufs=3) as opool,       # out f32
            tc.tile_pool(name="carry", bufs=4) as carry_pool,
            tc.tile_pool(name="bups", bufs=1, space="PSUM") as bu_ps,
            tc.tile_pool(name="myps", bufs=1, space="PSUM") as my_ps,
            tc.tile_pool(name="zps", bufs=2, space="PSUM") as z_ps,
        ):
            # ---- constants into SBUF
            xt = cpool.tile([128, t_len], f32r, tag="xt")
            for q in range(max(1, t_len // 2048)):
                w = min(2048, t_len)
                nc.sync.dma_start(xt[:, q * w:(q + 1) * w], xt_d[:, q * w:(q + 1) * w])
            beff = cpool.tile([128, 256], f32r, tag="beff")
            nc.sync.dma_start(beff[:], beff_d[:])
            pneg = cpool.tile([128, 2048], bf16, tag="pneg")
            nc.sync.dma_start(pneg[:], pneg_d[:])
            ppos2 = cpool.tile([128, 1024], bf16, tag="ppos2")
            nc.sync.dma_start(ppos2[:], ppos2_d[:])
            tri = cpool.tile([128, 512], bf16, tag="tri")
            nc.sync.dma_start(tri[:], tri_d[:])
            lamc = cpool.tile([128, 9], f32, tag="lamc")
            nc.sync.dma_start(lamc[:], lamc_d[:])
            ytw4 = cpool.tile([128, 512], bf16, tag="ytw4")
            nc.sync.dma_start(ytw4[:], ytw4_d[:])
            ytw_x = cpool.tile([128, 128], f32r, tag="ytwx")
            nc.sync.dma_start(ytw_x[:], ytwx_d[:])
            ident = cpool.tile([128, 128], f32r, tag="ident")
            nc.sync.dma_start(ident[:], ident_d[:])
            g1 = cpool.tile([128, 512], bf16, tag="g1")
            nc.sync.dma_start(g1[:], g1_d[:])
            g2 = cpool.tile([128, 4, 512], fp8, tag="g2")
            nc.sync.dma_start(g2[:], g2_d[:])
            g3 = cpool.tile([128, 4, 512], fp8, tag="g3")
            nc.sync.dma_start(g3[:], g3_d[:])
            g4 = cpool.tile([128, 4, 128], fp8, tag="g4")
            nc.sync.dma_start(g4[:], g4_d[:])
            bias0 = cpool.tile([128, 1], f32, tag="bias0")
            nc.vector.memset(bias0[:], 0.0)

            env.update(xt=xt, beff=beff, pneg=pneg, ppos2=ppos2, tri=tri,
                       lamc=lamc, ytw4=ytw4, ytw_x=ytw_x, ident=ident,
                       g1=g1, g2=g2, g3=g3, g4=g4, bias0=bias0,
                       bpool=bpool, epool=epool, postpool=postpool,
                       ypool=ypool, zw_pool=zw_pool, opool=opool,
                       carry_pool=carry_pool,
                       bu_ps=bu_ps, my_ps=my_ps, z_ps=z_ps)

            loop_cm = tc.For_i(0, reps) if reps > 1 else nullcontext()
            with loop_cm:
                _body(nc, tc, env)

    nc.finalize()
    return nc


def _body(nc, tc, env):
    import concourse.mybir as mybir
    from collections import deque
    f32 = mybir.dt.float32
    f32r = mybir.dt.float32r
    bf16 = mybir.dt.bfloat16
    fp8 = mybir.dt.float8e4
    AL = mybir.AluOpType
    ACT = mybir.ActivationFunctionType
    DR = mybir.MatmulPerfMode.DoubleRow

    (nchunk, out_d, xt, beff, pneg, ppos2, tri, lamc, ytw4, ytw_x, ident,
     g1, g2, g3, g4, bias0, bpool, epool, postpool, ypool, zw_pool, opool,
     carry_pool, bu_ps, my_ps, z_ps) = (
        env['nchunk'], env['out_d'], env['xt'], env['beff'], env['pneg'],
        env['ppos2'], env['tri'], env['lamc'], env['ytw4'], env['ytw_x'],
        env['ident'], env['g1'], env['g2'], env['g3'], env['g4'],
        env['bias0'], env['bpool'], env['epool'], env['postpool'],
        env['ypool'], env['zw_pool'], env['opool'], env['carry_pool'],
        env['bu_ps'], env['my_ps'], env['z_ps'])

    lam_re, lam_im, lam_imn = lamc[:, 0:1], lamc[:, 1:2], lamc[:, 2:3]
    lamL_re, lamL_im, lamL_imn = lamc[:, 3:4], lamc[:, 4:5], lamc[:, 5:6]
    lamL1_re, lamL1_im, lamL1_imn = lamc[:, 6:7], lamc[:, 7:8], lamc[:, 8:9]
    ppos2v = ppos2[:].rearrange("p (i n) -> p i n", i=2)

    state = dict(h_re=None, h_im=None, mp_re=None, mp_im=None)
    BU, BUSB, UP, MSB, PA, PB, YSB = {}, {}, {}, {}, {}, {}, {}
    Z1, Z2, Z3 = {}, {}, {}

    relu_rr = [0]
    # alternate ACT/DVE; ~half each
    def relu(dst, src_ps):
        if relu_rr[0] % 2 == 0:
            nc.scalar.activation(dst, src_ps, ACT.Relu, bias=bias0[:])
        else:
            nc.vector.tensor_scalar_max(dst, src_ps, 0.0)
        relu_rr[0] += 1

    # ---- stage S0: Bu matmuls (PE)
    def s_bu(c):
        t0 = c * L
        bu = bu_ps.tile([128, 1024], f32, tag="bu")
        for i in range(4):
            lhs = xt[:, t0 + i * 128: t0 + (i + 1) * 128]
            nc.tensor.matmul(bu[:, i * 256:(i + 1) * 256], lhs, beff[:],
                             start=True, stop=True)
        BU[c] = bu

    # ---- stage S1: psum->bf16, prescale products + folds
    def s_pre(c):
        bu = BU.pop(c)
        busb = bpool.tile([128, 1024], bf16, tag="busb")
        nc.scalar.activation(busb[:], bu[:], ACT.Identity, bias=bias0[:])
        e1 = epool.tile([128, 1024], bf16, tag="e1")
        e2 = epool.tile([128, 1024], bf16, tag="e2")
        nc.vector.tensor_tensor(e1[:], busb[:], pneg[:, 0:1024], AL.mult)
        nc.gpsimd.tensor_tensor(e2[:], busb[:], pneg[:, 1024:2048], AL.mult)
        e1v = e1[:].rearrange("p (i h n) -> p i h n", i=4, h=2)
        e2v = e2[:].rearrange("p (i h n) -> p i h n", i=4, h=2)
        up = epool.tile([128, 4, 256], bf16, tag="up")
        upv = up[:].rearrange("p i (h n) -> p i h n", h=2)
        nc.vector.tensor_tensor(upv[:, :, 0, :], e1v[:, :, 0, :], e1v[:, :, 1, :],
                                AL.subtract)
        nc.vector.tensor_tensor(upv[:, :, 1, :], e2v[:, :, 0, :], e2v[:, :, 1, :],
                                AL.add)
        UP[c] = up

    # ---- stage S2: carry ops, tri matmuls, mp, msb
    def s_tri(c):
        up = UP.pop(c)
        if c > 0:
            mp_re, mp_im = state['mp_re'], state['mp_im']
            h_re, h_im = state['h_re'], state['h_im']
            c1 = carry_pool.tile([128, 1], f32, tag="c1")
            h_re_n = carry_pool.tile([128, 1], f32, tag="hre")
            d1 = carry_pool.tile([128, 1], f32, tag="d1")
            h_im_n = carry_pool.tile([128, 1], f32, tag="him")
            nc.vector.scalar_tensor_tensor(c1[:], h_re[:], lamL_re, mp_re[:], AL.mult, AL.add)
            nc.vector.scalar_tensor_tensor(h_re_n[:], h_im[:], lamL_imn, c1[:], AL.mult, AL.add)
            nc.vector.scalar_tensor_tensor(d1[:], h_im[:], lamL_re, mp_im[:], AL.mult, AL.add)
            nc.vector.scalar_tensor_tensor(h_im_n[:], h_re[:], lamL_im, d1[:], AL.mult, AL.add)
            state['h_re'], state['h_im'] = h_re_n, h_im_n
            h_re, h_im = h_re_n, h_im_n
            a_re = carry_pool.tile([128, 1], f32, tag="are")
            a_im = carry_pool.tile([128, 1], f32, tag="aim")
            t1 = carry_pool.tile([128, 1], f32, tag="ct1")
            t2 = carry_pool.tile([128, 1], f32, tag="ct2")
            nc.vector.tensor_tensor(t1[:], h_im[:], lam_imn, AL.mult)
            nc.vector.scalar_tensor_tensor(a_re[:], h_re[:], lam_re, t1[:], AL.mult, AL.add)
            nc.vector.tensor_tensor(t2[:], h_re[:], lam_im, AL.mult)
            nc.vector.scalar_tensor_tensor(a_im[:], h_im[:], lam_re, t2[:], AL.mult, AL.add)
        else:
            a_re = carry_pool.tile([128, 1], f32, tag="are")
            a_im = carry_pool.tile([128, 1], f32, tag="aim")
            nc.vector.memset(a_re[:], 0.0)
            nc.vector.memset(a_im[:], 0.0)
            h0_re = carry_pool.tile([128, 1], f32, tag="hre")
            h0_im = carry_pool.tile([128, 1], f32, tag="him")
            nc.vector.memset(h0_re[:], 0.0)
            nc.vector.memset(h0_im[:], 0.0)
            state['h_re'], state['h_im'] = h0_re, h0_im
        m = my_ps.tile([128, 1024], f32, tag="my")
        for j in range(4):
            width = 512 - 128 * j
            nc.tensor.matmul(m[:, 128 * j:512], up[:, j, 0:128], tri[:, 0:width],
                             start=(j == 0), stop=(j == 3))
        for j in range(4):
            width = 512 - 128 * j
            nc.tensor.matmul(m[:, 512 + 128 * j:1024], up[:, j, 128:256],
                             tri[:, 0:width], start=(j == 0), stop=(j == 3))
        mr_col = m[:, 511:512]
        mi_col = m[:, 1023:1024]
        mp1 = carry_pool.tile([128, 1], f32, tag="mp1")
        mp_re = carry_pool.tile([128, 1], f32, tag="mpre")
        mp2 = carry_pool.tile([128, 1], f32, tag="mp2")
        mp_im = carry_pool.tile([128, 1], f32, tag="mpim")
        nc.vector.tensor_tensor(mp1[:], mr_col, lamL1_re, AL.mult)
        nc.vector.scalar_tensor_tensor(mp_re[:], mi_col, lamL1_imn, mp1[:], AL.mult, AL.add)
        nc.vector.tensor_tensor(mp2[:], mi_col, lamL1_re, AL.mult)
        nc.vector.scalar_tensor_tensor(mp_im[:], mr_col, lamL1_im, mp2[:], AL.mult, AL.add)
        state['mp_re'], state['mp_im'] = mp_re, mp_im
        msb = bpool.tile([128, 1024], bf16, tag="msb")
        nc.scalar.activation(msb[:, 0:512], m[:, 0:512], ACT.Identity, bias=a_re[:])
        nc.scalar.activation(msb[:, 512:1024], m[:, 512:1024], ACT.Identity, bias=a_im[:])
        MSB[c] = msb

    # ---- stage S3: postscale products, y matmuls, y_sb
    def s_post(c):
        t0 = c * L
        msb = MSB.pop(c)
        msb_re2 = msb[:, 0:512].unsqueeze(1).broadcast_to([128, 2, 512])
        msb_im2 = msb[:, 512:1024].unsqueeze(1).broadcast_to([128, 2, 512])
        postA = postpool.tile([128, 2, 512], bf16, tag="postA")
        postB = postpool.tile([128, 2, 512], bf16, tag="postB")
        nc.vector.tensor_tensor(postA[:], msb_re2, ppos2v, AL.mult)
        nc.gpsimd.tensor_tensor(postB[:], msb_im2, ppos2v, AL.mult)
        yp = z_ps.tile([128, 1024], f32, tag="zps", name="yp")
        y = yp[:, 0:512]
        nc.tensor.matmul(y, ytw4[:, 0:128], postA[:, 0, :], start=True, stop=False)
        nc.tensor.matmul(y, ytw4[:, 128:256], postA[:, 1, :], start=False, stop=False)
        nc.tensor.matmul(y, ytw4[:, 256:384], postB[:, 0, :], start=False, stop=False)
        nc.tensor.matmul(y, ytw4[:, 384:512], postB[:, 1, :], start=False, stop=False)
        nc.tensor.matmul(y, ytw_x[:], xt[:, t0:t0 + 512], start=False, stop=True)
        # y_sb written into the pair tile half (pair = c//2)
        pair = c // 2
        if pair not in YSB:
            YSB[pair] = ypool.tile([128, 1024], bf16, tag="ysb", name="ysb")
        nc.scalar.activation(YSB[pair][:, (c % 2) * 512:(c % 2) * 512 + 512], y,
                             ACT.Identity, bias=bias0[:])

    # ---- MLP stages (per quad q4: chunks 4q4 .. 4q4+3, pairs 2q4, 2q4+1)
    def m_g1(q4, which):      # which: 0 = pairA, 1 = pairB
        pair = 2 * q4 + which
        ysb = YSB.pop(pair)
        if q4 not in Z1:
            Z1[q4] = zw_pool.tile([128, 4, 2048], fp8, tag="z1w", name="z1w")
        z1w = Z1[q4]
        base = which * 1024
        for mi in range(4):
            zp = z_ps.tile([128, 1024], f32, tag="zps")
            gsl = g1[:, mi * 128:(mi + 1) * 128]
            nc.tensor.matmul(zp[:, 0:512], gsl, ysb[:, 0:512], start=True, stop=True)
            nc.tensor.matmul(zp[:, 512:1024], gsl, ysb[:, 512:1024], start=True, stop=True)
            relu(z1w[:, mi, base:base + 1024], zp[:])

    def m_dr(q4, gw, zin_d, zout_d):
        zin = zin_d.pop(q4)
        zout = zw_pool.tile([128, 4, 2048], fp8,
                            tag="z2w" if zout_d is Z2 else "z3w")
        zout_d[q4] = zout
        for mi in range(4):
            zpA = z_ps.tile([128, 1024], f32, tag="zps")
            zpB = z_ps.tile([128, 1024], f32, tag="zps")
            for q in range(2):
                gsl = gw[:, 2 * q:2 * q + 2, mi * 128:(mi + 1) * 128]
                for zp, base in ((zpA, 0), (zpB, 1024)):
                    for hh in range(2):
                        nc.tensor.matmul(
                            zp[:, hh * 512:(hh + 1) * 512],
                            gsl,
                            zin[:, 2 * q:2 * q + 2, base + hh * 512:base + (hh + 1) * 512],
                            start=(q == 0), stop=(q == 1),
                            perf_mode=DR, skip_group_check=True)
            relu(zout[:, mi, 0:1024], zpA[:])
            relu(zout[:, mi, 1024:2048], zpB[:])

    def m_g4(q4):
        z3w = Z3.pop(q4)
        t0 = 4 * q4 * L
        zpA = z_ps.tile([128, 1024], f32, tag="zps")
        zpB = z_ps.tile([128, 1024], f32, tag="zps")
        halves = [(zpA, 0, t0), (zpA, 1, t0 + L), (zpB, 0, t0 + 2 * L), (zpB, 1, t0 + 3 * L)]
        for q in range(2):
            gsl = g4[:, 2 * q:2 * q + 2, :]
            for i, (zp, hh, tx) in enumerate(halves):
                nc.tensor.matmul(zp[:, hh * 512:(hh + 1) * 512], gsl,
                                 z3w[:, 2 * q:2 * q + 2, i * 512:(i + 1) * 512],
                                 start=(q == 0), stop=False,
                                 perf_mode=DR, skip_group_check=True)
        for zp, hh, tx in halves:
            nc.tensor.matmul(zp[:, hh * 512:(hh + 1) * 512], ident[:],
                             xt[:, tx:tx + 512], start=False, stop=True,
                             skip_group_check=True)
        for zp, name in ((zpA, "oA"), (zpB, "oB")):
            o_sb = opool.tile([128, 1024], f32, tag="osb")
            nc.scalar.activation(o_sb[:], zp[:], ACT.Identity, bias=bias0[:])
            tx = t0 if name == "oA" else t0 + 2 * L
            nc.sync.dma_start(out_d[:, tx:tx + 512], o_sb[:, 0:512])
            nc.sync.dma_start(out_d[:, tx + 512:tx + 1024], o_sb[:, 512:1024])

    # ---- modulo-scheduled wave
    ablate = os.environ.get('KABLATE', '')
    mlp_q = deque()
    for t in range(nchunk + 4):
        c_post = t - 3
        c_tri = t - 2
        c_pre = t - 1
        if ablate == 'mlp':
            c = t - 3
            if 0 <= c < nchunk:
                pair = c // 2
                if pair not in YSB:
                    YSB[pair] = ypool.tile([128, 1024], bf16, tag="ysb", name="ysb")
                nc.vector.memset(YSB[pair][:, (c % 2) * 512:(c % 2) * 512 + 512], 0.01)
                if c % 4 == 3:
                    q4 = c // 4
                    mlp_q.append(lambda q4=q4: m_g1(q4, 0))
                    mlp_q.append(lambda q4=q4: m_g1(q4, 1))
                    mlp_q.append(lambda q4=q4: m_dr(q4, g2, Z1, Z2))
                    mlp_q.append(lambda q4=q4: m_dr(q4, g3, Z2, Z3))
                    mlp_q.append(lambda q4=q4: m_g4(q4))
            if mlp_q:
                mlp_q.popleft()()
            continue
        if 0 <= c_post < nchunk:
            s_post(c_post)
            if ablate == 'front' and c_post % 2 == 1:
                # keep out_d written; reuse y_sb as dummy output
                pair = c_post // 2
                ysb = YSB.pop(pair)
                nc.sync.dma_start(out_d[:, (c_post - 1) * L:(c_post - 1) * L + 512],
                                  ysb[:].bitcast(f32))
            if ablate != 'front' and c_post % 4 == 3:
                q4 = c_post // 4
                mlp_q.append(lambda q4=q4: m_g1(q4, 0))
                mlp_q.append(lambda q4=q4: m_g1(q4, 1))
                mlp_q.append(lambda q4=q4: m_dr(q4, g2, Z1, Z2))
                mlp_q.append(lambda q4=q4: m_dr(q4, g3, Z2, Z3))
                mlp_q.append(lambda q4=q4: m_g4(q4))
        if 0 <= c_tri < nchunk:
            s_tri(c_tri)
        if 0 <= c_pre < nchunk:
            s_pre(c_pre)
        if t < nchunk:
            s_bu(t)
        if mlp_q:
            mlp_q.popleft()()
    while mlp_q:
        mlp_q.popleft()()
    if ablate == 'mlp':
        pass


# ---------------------------------------------------------------- PJRT runner

def _make_runner(nc, n_cores):
    import jax
    from jax.sharding import Mesh, PartitionSpec
    from jax.experimental.shard_map import shard_map
    import concourse.mybir as mybir
    from concourse import bass2jax

    bass2jax.install_neuronx_cc_hook()
    assert nc.is_finalized()
    partition_name = nc.partition_id_tensor.name if nc.partition_id_tensor else None

    in_names, out_names, out_avals, zero_shapes = [], [], [], []
    for alloc in nc.m.functions[0].allocations:
        if not isinstance(alloc, mybir.MemoryLocationSet):
            continue
        name = alloc.memorylocations[0].name
        if alloc.kind == "ExternalInput":
            if name != partition_name:
                in_names.append(name)
        elif alloc.kind == "ExternalOutput":
            shape = tuple(alloc.tensor_shape)
            dtype = mybir.dt.np(alloc.dtype)
            out_names.append(name)
            out_avals.append(jax.core.ShapedArray(shape, dtype))
            zero_shapes.append((shape, dtype))
    n_params = len(in_names)
    n_outs = len(out_avals)
    all_in_names = list(in_names) + list(out_names)
    if partition_name is not None:
        all_in_names.append(partition_name)
    donate = tuple(range(n_params, n_params + n_outs))

    def _body_fn(*args):
        operands = list(args)
        if partition_name is not None:
            operands.append(bass2jax.partition_id_tensor())
        outs = bass2jax._bass_exec_p.bind(
            *operands,
            out_avals=tuple(out_avals),
            in_names=tuple(all_in_names),
            out_names=tuple(out_names),
            lowering_input_output_aliases=(),
            sim_require_finite=True,
            sim_require_nnan=True,
            nc=nc,
        )
        return tuple(outs)

    devices = jax.devices()[:n_cores]
    if n_cores == 1:
        fn = jax.jit(_body_fn, donate_argnums=donate, keep_unused=True)
    else:
        mesh = Mesh(np.asarray(devices), ("core",))
        fn = jax.jit(
            shard_map(_body_fn, mesh=mesh,
                      in_specs=(PartitionSpec("core"),) * (n_params + n_outs),
                      out_specs=(PartitionSpec("core"),) * n_outs,
                      check_rep=False),
            donate_argnums=donate, keep_unused=True,
        )

    def run(per_core_inputs):
        if n_cores == 1:
            ins = [np.asarray(per_core_inputs[0][n]) for n in in_names]
            zeros = [np.zeros(s, d) for s, d in zero_shapes]
        else:
            ins = [np.concatenate([np.asarray(per_core_inputs[c][n])
                                   for c in range(n_cores)], axis=0) for n in in_names]
            zeros = [np.zeros((n_cores * s[0], *s[1:]), d) for s, d in zero_shapes]
        out_arrs = fn(*ins, *zeros)
        if n_cores == 1:
            return [{name: np.asarray(out_arrs[i]) for i, name in enumerate(out_names)}]
        res = []
        for c in range(n_cores):
            d = {}
            for i, name in enumerate(out_names):
                full = np.asarray(out_arrs[i])
                d[name] = full.reshape(n_cores, *out_avals[i].shape)[c]
            res.append(d)
        return res

    run.fn = fn
    run.in_names = in_names
    run.out_names = out_names
    run.zero_shapes = zero_shapes
    return run


_RUNNER = None


def _get_runner():
    global _RUNNER
    if _RUNNER is None:
        nc = _build_program(T)
        _RUNNER = _make_runner(nc, NCORES)
    return _RUNNER


def kernel(**inputs):
    import time as _time
    global _RUNNER
    p = {k: np.asarray(v) for k, v in inputs.items()}
    consts = _host_prep(p)
    x = p['x'].astype(np.float32)            # [B, T, D]
    per_core = []
    for b in range(B):
        m = dict(consts)
        m['xt'] = np.ascontiguousarray(x[b].T)
        per_core.append(m)
    res = None
    for attempt in range(3):
        try:
            run = _get_runner()
            res = run(per_core)
            break
        except Exception:
            # transient NRT exec faults have been observed on the first
            # execution of a freshly compiled NEFF; rebuild the jitted
            # callable (NEFF comes from the compile cache) and retry.
            _RUNNER = None
            if attempt == 2:
                raise
            _time.sleep(2.0)
    out = np.stack([res[b]['outT'].T for b in range(B)], axis=0)
    return np.ascontiguousarray(out, dtype=np.float32)


# revision 14
# speedup vs baseline: 1.2137x; 1.0098x over previous
"""Trainium2 Bass kernel for nn_DWNBlock (LRU scan + Lipschitz sandwich MLP).

v3: tri-matmul chunked scan (PE cumsum) + packed rotation products +
5-matmul y fold + chunk-paired fp8-DR MLP.

Per core (batch-parallel, x^T channel-major [128, 8192]):
  1. Bu in [s, n] layout via PE (f32r): psum [128,1024], s-tile i at
     cols 256i..256i+256 = [re n | im n]
  2. prescale by lam^-s (bf16 packed tables, DVE/Pool), shared
     upper-tri-ones matmuls (PE, bf16) -> m[n, t] psum (cumsum +
     transpose in one), carry folded as per-partition bias in the
     psum->bf16 copy (ACT), tiny f32 carry chain ops (DVE)
  3. postscale: packed [pos_re|pos_im] x [m|m] stride-0 products;
     y = C_re@H_re - C_im@H_im + D@x as 5 accumulated matmuls on the
     products directly (H never materialized)
  4. folded MLP: relu(G1 y) bf16 -> fp8 -> G2/G3/G4 fp8 DoubleRow,
     chunk-PAIRED so each DR weight load covers 2 matmuls; residual +x
     folded into the G4 psum group via an identity matmul
Precision: same scan math as the proven baseline (~1.3e-2 vs 2e-2 gate).
"""
import math
import os
import sys

for _p in ('/opt/trn_rl_repo',):
    if _p not in sys.path:
        sys.path.insert(0, _p)

import numpy as np
import ml_dtypes

D = 128          # d_model
N = 128          # d_state
H = 512          # MLP hidden
T = 8192         # sequence length
B = 8            # batch
L = 512          # scan chunk length
NCORES = 8
SCALE = 1.0
SQRT2 = math.sqrt(2.0)

F8NP = ml_dtypes.float8_e4m3
BFNP = ml_dtypes.bfloat16


# ---------------------------------------------------------------- host prep

def _cayley64(W):
    cout, cin = W.shape
    if cin > cout:
        return _cayley64(W.T).T
    U, V = W[:cin], W[cin:]
    I = np.eye(cin, dtype=W.dtype)
    A = U - U.T + V.T @ V
    iIpA = np.linalg.inv(I + A)
    return np.concatenate([iIpA @ (I - A), -2.0 * V @ iIpA], axis=0)


def _host_prep(p):
    """Fold all parameters into device constants."""
    f8 = np.float64
    nu_log = p['nu_log'].astype(f8)
    theta_log = p['theta_log'].astype(f8)
    gamma_log = p['gamma_log'].astype(f8)
    lam = np.exp(-np.exp(nu_log)) * np.exp(1j * np.exp(theta_log))   # [N]
    Beff = np.exp(gamma_log)[:, None] * (p['B_re'].astype(f8) + 1j * p['B_im'].astype(f8))
    beff_w = np.concatenate([Beff.real.T, Beff.imag.T], axis=1)      # [D, 2N]

    s = np.arange(L)
    loglam = np.log(lam)
    pneg = np.exp(-s[:, None] * loglam[None, :])    # [L, N] = lam^-s
    ppos = np.exp(s[None, :] * loglam[:, None])     # [N, L] = lam^t
    lamL = lam ** L
    lamL1 = lam ** (L - 1)

    # Bu psum layout [128, 1024]: s-tile i at [256i:256(i+1)] = [re n | im n],
    # partition = local time p (global s = 128 i + p).
    def _pk(i):   # s-tile i of pneg, [128, N]
        return pneg[i * 128:(i + 1) * 128, :]
    pnegA = np.concatenate(sum([[_pk(i).real, _pk(i).imag] for i in range(4)], []), axis=1)
    pnegB = np.concatenate(sum([[_pk(i).imag, _pk(i).real] for i in range(4)], []), axis=1)
    pneg_pack = np.concatenate([pnegA, pnegB], axis=1)               # [128, 2048]

    ppos2 = np.concatenate([ppos.real, ppos.imag], axis=1)           # [128, 1024]

    # tri_ones[s, u] = 1 if s <= u, over [128, 512]
    tri_ones = (np.arange(128)[:, None] <= np.arange(512)[None, :]).astype(f8)

    # carry-chain per-partition scalar columns
    lamcols = np.stack([
        lam.real, lam.imag, -lam.imag,
        lamL.real, lamL.imag, -lamL.imag,
        lamL1.real, lamL1.imag, -lamL1.imag,
    ], axis=1)                                       # [128, 9]

    C = p['C_re'].astype(f8) + 1j * p['C_im'].astype(f8)             # [D, N]
    # postA = [pos_re*m_re | pos_im*m_re] = [P1|P3]
    # postB = [pos_re*m_im | pos_im*m_im] = [P4|P2]
    # y = C_re@P1 - C_im@P3 - C_im@P4 - C_re@P2
    ytw4 = np.concatenate([C.real.T, -C.imag.T, -C.imag.T, -C.real.T], axis=1)  # [N, 4D]
    ytw_x = np.ascontiguousarray(p['Dmat'].astype(f8).T)             # [128, 128]
    ident = np.eye(128, dtype=f8)

    def _q(Wkey, akey, fout):
        Wd = p[Wkey].astype(f8)
        Q = _cayley64((float(p[akey][0]) / np.linalg.norm(Wd)) * Wd)
        return Q[:, fout:], Q[:, :fout]

    Q1in, Q1out = _q('W1', 'alpha1', H)
    Q2in, Q2out = _q('W2', 'alpha2', H)
    Q3in, Q3out = _q('W3', 'alpha3', H)
    Qlin = _cayley64((float(p['alphal'][0]) / np.linalg.norm(p['Wl'].astype(f8)))
                     * p['Wl'].astype(f8))[:, D:]    # [128, 512]

    e = np.exp
    ps1, ps2, ps3 = p['psi1'].astype(f8), p['psi2'].astype(f8), p['psi3'].astype(f8)
    G1 = SCALE * SCALE * SQRT2 * (Q1in.T * e(-ps1)[None, :])                    # [128, 512]
    G2 = 2.0 * SCALE * (e(ps1)[:, None] * Q1out) @ (Q2in.T * e(-ps2)[None, :])  # [512, 512]
    G3 = 2.0 * SCALE * (e(ps2)[:, None] * Q2out) @ (Q3in.T * e(-ps3)[None, :])  # [512, 512]
    G4 = SQRT2 * SCALE * (e(ps3)[:, None] * Q3out) @ Qlin.T                     # [512, 128]

    def pack_kt(G, width):     # [K, width] -> [128, K//128 * width] k-tile-major
        K = G.shape[0]
        return np.concatenate([G[k * 128:(k + 1) * 128, :] for k in range(K // 128)], axis=1)

    out32 = dict(beff_w=beff_w, lamcols=lamcols, ytw_x=ytw_x, ident=ident)
    outbf = dict(pneg_pack=pneg_pack, ppos2=ppos2, tri_ones=tri_ones,
                 ytw4=ytw4, g1=G1)
    outf8 = dict(g2=pack_kt(G2, 512), g3=pack_kt(G3, 512), g4=pack_kt(G4, 128))
    res = {k: np.ascontiguousarray(v, dtype=np.float32) for k, v in out32.items()}
    res.update({k: np.ascontiguousarray(v.astype(np.float32), dtype=BFNP)
                for k, v in outbf.items()})
    res.update({k: np.ascontiguousarray(v.astype(np.float32), dtype=F8NP)
                for k, v in outf8.items()})
    return res


# ---------------------------------------------------------------- device program

def _build_program(t_len, reps=1):
    from contextlib import nullcontext
    from concourse import bacc
    import concourse.mybir as mybir
    from concourse.tile import TileContext

    f32 = mybir.dt.float32
    f32r = mybir.dt.float32r
    bf16 = mybir.dt.bfloat16
    fp8 = mybir.dt.float8e4
    nchunk = t_len // L

    nc = bacc.Bacc("TRN2", target_bir_lowering=False, debug=False)

    xt_d = nc.dram_tensor("xt", [128, t_len], f32r, kind="ExternalInput").ap()
    beff_d = nc.dram_tensor("beff_w", [128, 256], f32r, kind="ExternalInput").ap()
    pneg_d = nc.dram_tensor("pneg_pack", [128, 2048], bf16, kind="ExternalInput").ap()
    ppos2_d = nc.dram_tensor("ppos2", [128, 1024], bf16, kind="ExternalInput").ap()
    tri_d = nc.dram_tensor("tri_ones", [128, 512], bf16, kind="ExternalInput").ap()
    lamc_d = nc.dram_tensor("lamcols", [128, 9], f32, kind="ExternalInput").ap()
    ytw4_d = nc.dram_tensor("ytw4", [128, 512], bf16, kind="ExternalInput").ap()
    ytwx_d = nc.dram_tensor("ytw_x", [128, 128], f32r, kind="ExternalInput").ap()
    ident_d = nc.dram_tensor("ident", [128, 128], f32r, kind="ExternalInput").ap()
    g1_d = nc.dram_tensor("g1", [128, 512], bf16, kind="ExternalInput").ap()
    g2_d = nc.dram_tensor("g2", [128, 2048], fp8, kind="ExternalInput").ap()
    g3_d = nc.dram_tensor("g3", [128, 2048], fp8, kind="ExternalInput").ap()
    g4_d = nc.dram_tensor("g4", [128, 512], fp8, kind="ExternalInput").ap()
    out_d = nc.dram_tensor("outT", [128, t_len], f32, kind="ExternalOutput").ap()

    env = dict(nchunk=nchunk, out_d=out_d)

    with TileContext(nc) as tc:
        with (
            tc.tile_pool(name="const", bufs=1) as cpool,
            tc.tile_pool(name="bpool", bufs=3) as bpool,       # busb bf16
            tc.tile_pool(name="epool", bufs=3) as epool,       # e1/e2
            tc.tile_pool(name="postpool", bufs=3) as postpool, # postA/postB
            tc.tile_pool(name="ypool", bufs=4) as ypool,       # y_sb bf16
            tc.tile_pool(name="zw", bufs=2) as zw_pool,        # fp8 activations
            tc.tile_pool(name="opool", bufs=3) as opool,       # out f32
            tc.tile_pool(name="carry", bufs=4) as carry_pool,
            tc.tile_pool(name="bups", bufs=1, space="PSUM") as bu_ps,
            tc.tile_pool(name="myps", bufs=1, space="PSUM") as my_ps,
            tc.tile_pool(name="zps", bufs=2, space="PSUM") as z_ps,
        ):
            # ---- constants into SBUF
            xt = cpool.tile([128, t_len], f32r, tag="xt")
            for q in range(max(1, t_len // 2048)):
                w = min(2048, t_len)
                nc.sync.dma_start(xt[:, q * w:(q + 1) * w], xt_d[:, q * w:(q + 1) * w])
            beff = cpool.tile([128, 256], f32r, tag="beff")
            nc.sync.dma_start(beff[:], beff_d[:])
            pneg = cpool.tile([128, 2048], bf16, tag="pneg")
            nc.sync.dma_start(pneg[:], pneg_d[:])
            ppos2 = cpool.tile([128, 1024], bf16, tag="ppos2")
            nc.sync.dma_start(ppos2[:], ppos2_d[:])
            tri = cpool.tile([128, 512], bf16, tag="tri")
            nc.sync.dma_start(tri[:], tri_d[:])
            lamc = cpool.tile([128, 9], f32, tag="lamc")
            nc.sync.dma_start(lamc[:], lamc_d[:])
            ytw4 = cpool.tile([128, 512], bf16, tag="ytw4")
            nc.sync.dma_start(ytw4[:], ytw4_d[:])
            ytw_x = cpool.tile([128, 128], f32r, tag="ytwx")
            nc.sync.dma_start(ytw_x[:], ytwx_d[:])
            ident = cpool.tile([128, 128], f32r, tag="ident")
            nc.sync.dma_start(ident[:], ident_d[:])
            g1 = cpool.tile([128, 512], bf16, tag="g1")
            nc.sync.dma_start(g1[:], g1_d[:])
            g2 = cpool.tile([128, 4, 512], fp8, tag="g2")
            nc.sync.dma_start(g2[:], g2_d[:])
            g3 = cpool.tile([128, 4, 512], fp8, tag="g3")
            nc.sync.dma_start(g3[:], g3_d[:])
            g4 = cpool.tile([128, 4, 128], fp8, tag="g4")
            nc.sync.dma_start(g4[:], g4_d[:])
            bias0 = cpool.tile([128, 1], f32, tag="bias0")
            nc.vector.memset(bias0[:], 0.0)

            env.update(xt=xt, beff=beff, pneg=pneg, ppos2=ppos2, tri=tri,
                       lamc=lamc, ytw4=ytw4, ytw_x=ytw_x, ident=ident,
                       g1=g1, g2=g2, g3=g3, g4=g4, bias0=bias0,
                       bpool=bpool, epool=epool, postpool=postpool,
                       ypool=ypool, zw_pool=zw_pool, opool=opool,
                       carry_pool=carry_pool,
                       bu_ps=bu_ps, my_ps=my_ps, z_ps=z_ps)

            loop_cm = tc.For_i(0, reps) if reps > 1 else nullcontext()
            with loop_cm:
                _body(nc, tc, env)

    nc.finalize()
    return nc


def _body(nc, tc, env):
    import concourse.mybir as mybir
    from collections import deque
    f32 = mybir.dt.float32
    f32r = mybir.dt.float32r
    bf16 = mybir.dt.bfloat16
    fp8 = mybir.dt.float8e4
    AL = mybir.AluOpType
    ACT = mybir.ActivationFunctionType
    DR = mybir.MatmulPerfMode.DoubleRow

    (nchunk, out_d, xt, beff, pneg, ppos2, tri, lamc, ytw4, ytw_x, ident,
     g1, g2, g3, g4, bias0, bpool, epool, postpool, ypool, zw_pool, opool,
     carry_pool, bu_ps, my_ps, z_ps) = (
        env['nchunk'], env['out_d'], env['xt'], env['beff'], env['pneg'],
        env['ppos2'], env['tri'], env['lamc'], env['ytw4'], env['ytw_x'],
        env['ident'], env['g1'], env['g2'], env['g3'], env['g4'],
        env['bias0'], env['bpool'], env['epool'], env['postpool'],
        env['ypool'], env['zw_pool'], env['opool'], env['carry_pool'],
        env['bu_ps'], env['my_ps'], env['z_ps'])

    lam_re, lam_im, lam_imn = lamc[:, 0:1], lamc[:, 1:2], lamc[:, 2:3]
    lamL_re, lamL_im, lamL_imn = lamc[:, 3:4], lamc[:, 4:5], lamc[:, 5:6]
    lamL1_re, lamL1_im, lamL1_imn = lamc[:, 6:7], lamc[:, 7:8], lamc[:, 8:9]
    ppos2v = ppos2[:].rearrange("p (i n) -> p i n", i=2)

    state = dict(h_re=None, h_im=None, mp_re=None, mp_im=None)
    BU, BUSB, UP, MSB, PA, PB, YSB = {}, {}, {}, {}, {}, {}, {}
    Z1, Z2, Z3 = {}, {}, {}

    relu_rr = [0]
    # alternate ACT/DVE; ~half each
    def relu(dst, src_ps):
        if relu_rr[0] % 2 == 0:
            nc.scalar.activation(dst, src_ps, ACT.Relu, bias=bias0[:])
        else:
            nc.vector.tensor_scalar_max(dst, src_ps, 0.0)
        relu_rr[0] += 1

    # ---- stage S0: Bu matmuls (PE)
    def s_bu(c):
        t0 = c * L
        bu = bu_ps.tile([128, 1024], f32, tag="bu")
        for i in range(4):
            lhs = xt[:, t0 + i * 128: t0 + (i + 1) * 128]
            nc.tensor.matmul(bu[:, i * 256:(i + 1) * 256], lhs, beff[:],
                             start=True, stop=True)
        BU[c] = bu

    # ---- stage S1: psum->bf16, prescale products + folds
    def s_pre(c):
        bu = BU.pop(c)
        busb = bpool.tile([128, 1024], bf16, tag="busb")
        nc.scalar.activation(busb[:], bu[:], ACT.Identity, bias=bias0[:])
        e1 = epool.tile([128, 1024], bf16, tag="e1")
        e2 = epool.tile([128, 1024], bf16, tag="e2")
        nc.vector.tensor_tensor(e1[:], busb[:], pneg[:, 0:1024], AL.mult)
        nc.gpsimd.tensor_tensor(e2[:], busb[:], pneg[:, 1024:2048], AL.mult)
        e1v = e1[:].rearrange("p (i h n) -> p i h n", i=4, h=2)
        e2v = e2[:].rearrange("p (i h n) -> p i h n", i=4, h=2)
        up = epool.tile([128, 4, 256], bf16, tag="up")
        upv = up[:].rearrange("p i (h n) -> p i h n", h=2)
        nc.vector.tensor_tensor(upv[:, :, 0, :], e1v[:, :, 0, :], e1v[:, :, 1, :],
                                AL.subtract)
        nc.vector.tensor_tensor(upv[:, :, 1, :], e2v[:, :, 0, :], e2v[:, :, 1, :],
                                AL.add)
        UP[c] = up

    # ---- stage S2: carry ops, tri matmuls, mp, msb
    def s_tri(c):
        up = UP.pop(c)
        if c > 0:
            mp_re, mp_im = state['mp_re'], state['mp_im']
            h_re, h_im = state['h_re'], state['h_im']
            c1 = carry_pool.tile([128, 1], f32, tag="c1")
            h_re_n = carry_pool.tile([128, 1], f32, tag="hre")
            d1 = carry_pool.tile([128, 1], f32, tag="d1")
            h_im_n = carry_pool.tile([128, 1], f32, tag="him")
            nc.vector.scalar_tensor_tensor(c1[:], h_re[:], lamL_re, mp_re[:], AL.mult, AL.add)
            nc.vector.scalar_tensor_tensor(h_re_n[:], h_im[:], lamL_imn, c1[:], AL.mult, AL.add)
            nc.vector.scalar_tensor_tensor(d1[:], h_im[:], lamL_re, mp_im[:], AL.mult, AL.add)
            nc.vector.scalar_tensor_tensor(h_im_n[:], h_re[:], lamL_im, d1[:], AL.mult, AL.add)
            state['h_re'], state['h_im'] = h_re_n, h_im_n
            h_re, h_im = h_re_n, h_im_n
            a_re = carry_pool.tile([128, 1], f32, tag="are")
            a_im = carry_pool.tile([128, 1], f32, tag="aim")
            t1 = carry_pool.tile([128, 1], f32, tag="ct1")
            t2 = carry_pool.tile([128, 1], f32, tag="ct2")
            nc.vector.tensor_tensor(t1[:], h_im[:], lam_imn, AL.mult)
            nc.vector.scalar_tensor_tensor(a_re[:], h_re[:], lam_re, t1[:], AL.mult, AL.add)
            nc.vector.tensor_tensor(t2[:], h_re[:], lam_im, AL.mult)
            nc.vector.scalar_tensor_tensor(a_im[:], h_im[:], lam_re, t2[:], AL.mult, AL.add)
        else:
            a_re = carry_pool.tile([128, 1], f32, tag="are")
            a_im = carry_pool.tile([128, 1], f32, tag="aim")
            nc.vector.memset(a_re[:], 0.0)
            nc.vector.memset(a_im[:], 0.0)
            h0_re = carry_pool.tile([128, 1], f32, tag="hre")
            h0_im = carry_pool.tile([128, 1], f32, tag="him")
            nc.vector.memset(h0_re[:], 0.0)
            nc.vector.memset(h0_im[:], 0.0)
            state['h_re'], state['h_im'] = h0_re, h0_im
        m = my_ps.tile([128, 1024], f32, tag="my")
        for j in range(4):
            width = 512 - 128 * j
            nc.tensor.matmul(m[:, 128 * j:512], up[:, j, 0:128], tri[:, 0:width],
                             start=(j == 0), stop=(j == 3))
        for j in range(4):
            width = 512 - 128 * j
            nc.tensor.matmul(m[:, 512 + 128 * j:1024], up[:, j, 128:256],
                             tri[:, 0:width], start=(j == 0), stop=(j == 3))
        mr_col = m[:, 511:512]
        mi_col = m[:, 1023:1024]
        mp1 = carry_pool.tile([128, 1], f32, tag="mp1")
        mp_re = carry_pool.tile([128, 1], f32, tag="mpre")
        mp2 = carry_pool.tile([128, 1], f32, tag="mp2")
        mp_im = carry_pool.tile([128, 1], f32, tag="mpim")
        nc.vector.tensor_tensor(mp1[:], mr_col, lamL1_re, AL.mult)
        nc.vector.scalar_tensor_tensor(mp_re[:], mi_col, lamL1_imn, mp1[:], AL.mult, AL.add)
        nc.vector.tensor_tensor(mp2[:], mi_col, lamL1_re, AL.mult)
        nc.vector.scalar_tensor_tensor(mp_im[:], mr_col, lamL1_im, mp2[:], AL.mult, AL.add)
        state['mp_re'], state['mp_im'] = mp_re, mp_im
        msb = bpool.tile([128, 1024], bf16, tag="msb")
        nc.scalar.activation(msb[:, 0:512], m[:, 0:512], ACT.Identity, bias=a_re[:])
        nc.scalar.activation(msb[:, 512:1024], m[:, 512:1024], ACT.Identity, bias=a_im[:])
        MSB[c] = msb

    # ---- stage S3: postscale products, y matmuls, y_sb
    def s_post(c):
        t0 = c * L
        msb = MSB.pop(c)
        msb_re2 = msb[:, 0:512].unsqueeze(1).broadcast_to([128, 2, 512])
        msb_im2 = msb[:, 512:1024].unsqueeze(1).broadcast_to([128, 2, 512])
        postA = postpool.tile([128, 2, 512], bf16, tag="postA")
        postB = postpool.tile([128, 2, 512], bf16, tag="postB")
        nc.vector.tensor_tensor(postA[:], msb_re2, ppos2v, AL.mult)
        nc.gpsimd.tensor_tensor(postB[:], msb_im2, ppos2v, AL.mult)
        yp = z_ps.tile([128, 1024], f32, tag="zps", name="yp")
        y = yp[:, 0:512]
        nc.tensor.matmul(y, ytw4[:, 0:128], postA[:, 0, :], start=True, stop=False)
        nc.tensor.matmul(y, ytw4[:, 128:256], postA[:, 1, :], start=False, stop=False)
        nc.tensor.matmul(y, ytw4[:, 256:384], postB[:, 0, :], start=False, stop=False)
        nc.tensor.matmul(y, ytw4[:, 384:512], postB[:, 1, :], start=False, stop=False)
        nc.tensor.matmul(y, ytw_x[:], xt[:, t0:t0 + 512], start=False, stop=True)
        # y_sb written into the pair tile half (pair = c//2)
        pair = c // 2
        if pair not in YSB:
            YSB[pair] = ypool.tile([128, 1024], bf16, tag="ysb", name="ysb")
        nc.scalar.activation(YSB[pair][:, (c % 2) * 512:(c % 2) * 512 + 512], y,
                             ACT.Identity, bias=bias0[:])

    # ---- MLP stages (per quad q4: chunks 4q4 .. 4q4+3, pairs 2q4, 2q4+1)
    def m_g1(q4, which):      # which: 0 = pairA, 1 = pairB
        pair = 2 * q4 + which
        ysb = YSB.pop(pair)
        if q4 not in Z1:
            Z1[q4] = zw_pool.tile([128, 4, 2048], fp8, tag="z1w", name="z1w")
        z1w = Z1[q4]
        base = which * 1024
        for mi in range(4):
            zp = z_ps.tile([128, 1024], f32, tag="zps")
            gsl = g1[:, mi * 128:(mi + 1) * 128]
            nc.tensor.matmul(zp[:, 0:512], gsl, ysb[:, 0:512], start=True, stop=True)
            nc.tensor.matmul(zp[:, 512:1024], gsl, ysb[:, 512:1024], start=True, stop=True)
            relu(z1w[:, mi, base:base + 1024], zp[:])

    def m_dr(q4, gw, zin_d, zout_d):
        zin = zin_d.pop(q4)
        zout = zw_pool.tile([128, 4, 2048], fp8,
                            tag="z2w" if zout_d is Z2 else "z3w")
        zout_d[q4] = zout
        for mi in range(4):
            zpA = z_ps.tile([128, 1024], f32, tag="zps")
            zpB = z_ps.tile([128, 1024], f32, tag="zps")
            for q in range(2):
                gsl = gw[:, 2 * q:2 * q + 2, mi * 128:(mi + 1) * 128]
                for zp, base in ((zpA, 0), (zpB, 1024)):
                    for hh in range(2):
                        nc.tensor.matmul(
                            zp[:, hh * 512:(hh + 1) * 512],
                            gsl,
                            zin[:, 2 * q:2 * q + 2, base + hh * 512:base + (hh + 1) * 512],
                            start=(q == 0), stop=(q == 1),
                            perf_mode=DR, skip_group_check=True)
            relu(zout[:, mi, 0:1024], zpA[:])
            relu(zout[:, mi, 1024:2048], zpB[:])

    def m_g4(q4):
        z3w = Z3.pop(q4)
        t0 = 4 * q4 * L
        zpA = z_ps.tile([128, 1024], f32, tag="zps")
        zpB = z_ps.tile([128, 1024], f32, tag="zps")
        halves = [(zpA, 0, t0), (zpA, 1, t0 + L), (zpB, 0, t0 + 2 * L), (zpB, 1, t0 + 3 * L)]
        for q in range(2):
            gsl = g4[:, 2 * q:2 * q + 2, :]
            for i, (zp, hh, tx) in enumerate(halves):
                nc.tensor.matmul(zp[:, hh * 512:(hh + 1) * 512], gsl,
                                 z3w[:, 2 * q:2 * q + 2, i * 512:(i + 1) * 512],
                                 start=(q == 0), stop=False,
                                 perf_mode=DR, skip_group_check=True)
        for zp, hh, tx in halves:
            nc.tensor.matmul(zp[:, hh * 512:(hh + 1) * 512], ident[:],
                             xt[:, tx:tx + 512], start=False, stop=True,
                             skip_group_check=True)
        for zp, name in ((zpA, "oA"), (zpB, "oB")):
            o_sb = opool.tile([128, 1024], f32, tag="osb")
            nc.scalar.activation(o_sb[:], zp[:], ACT.Identity, bias=bias0[:])
            tx = t0 if name == "oA" else t0 + 2 * L
            nc.sync.dma_start(out_d[:, tx:tx + 512], o_sb[:, 0:512])
            nc.sync.dma_start(out_d[:, tx + 512:tx + 1024], o_sb[:, 512:1024])

    # ---- modulo-scheduled wave
    ablate = os.environ.get('KABLATE', '')
    mlp_q = deque()
    for t in range(nchunk + 4):
        c_post = t - 3
        c_tri = t - 2
        c_pre = t - 1
        if ablate == 'mlp':
            c = t - 3
            if 0 <= c < nchunk:
                pair = c // 2
                if pair not in YSB:
                    YSB[pair] = ypool.tile([128, 1024], bf16, tag="ysb", name="ysb")
                nc.vector.memset(YSB[pair][:, (c % 2) * 512:(c % 2) * 512 + 512], 0.01)
                if c % 4 == 3:
                    q4 = c // 4
                    mlp_q.append(lambda q4=q4: m_g1(q4, 0))
                    mlp_q.append(lambda q4=q4: m_g1(q4, 1))
                    mlp_q.append(lambda q4=q4: m_dr(q4, g2, Z1, Z2))
                    mlp_q.append(lambda q4=q4: m_dr(q4, g3, Z2, Z3))
                    mlp_q.append(lambda q4=q4: m_g4(q4))
            if mlp_q:
                mlp_q.popleft()()
            continue
        if 0 <= c_post < nchunk:
            s_post(c_post)
            if ablate == 'front' and c_post % 2 == 1:
                # keep out_d written; reuse y_sb as dummy output
                pair = c_post // 2
                ysb = YSB.pop(pair)
                nc.sync.dma_start(out_d[:, (c_post - 1) * L:(c_post - 1) * L + 512],
                                  ysb[:].bitcast(f32))
            if ablate != 'front' and c_post % 4 == 3:
                q4 = c_post // 4
                mlp_q.append(lambda q4=q4: m_g1(q4, 0))
                mlp_q.append(lambda q4=q4: m_g1(q4, 1))
                mlp_q.append(lambda q4=q4: m_dr(q4, g2, Z1, Z2))
                mlp_q.append(lambda q4=q4: m_dr(q4, g3, Z2, Z3))
                mlp_q.append(lambda q4=q4: m_g4(q4))
        if 0 <= c_tri < nchunk:
            s_tri(c_tri)
        if 0 <= c_pre < nchunk:
            s_pre(c_pre)
        if t < nchunk:
            s_bu(t)
        if mlp_q:
            mlp_q.popleft()()
        if mlp_q and t % 4 == 3:
            mlp_q.popleft()()
    while mlp_q:
        mlp_q.popleft()()
    if ablate == 'mlp':
        pass


# ---------------------------------------------------------------- PJRT runner

def _make_runner(nc, n_cores):
    import jax
    from jax.sharding import Mesh, PartitionSpec
    from jax.experimental.shard_map import shard_map
    import concourse.mybir as mybir
    from concourse import bass2jax

    bass2jax.install_neuronx_cc_hook()
    assert nc.is_finalized()
    partition_name = nc.partition_id_tensor.name if nc.partition_id_tensor else None

    in_names, out_names, out_avals, zero_shapes = [], [], [], []
    for alloc in nc.m.functions[0].allocations:
        if not isinstance(alloc, mybir.MemoryLocationSet):
            continue
        name = alloc.memorylocations[0].name
        if alloc.kind == "ExternalInput":
            if name != partition_name:
                in_names.append(name)
        elif alloc.kind == "ExternalOutput":
            shape = tuple(alloc.tensor_shape)
            dtype = mybir.dt.np(alloc.dtype)
            out_names.append(name)
            out_avals.append(jax.core.ShapedArray(shape, dtype))
            zero_shapes.append((shape, dtype))
    n_params = len(in_names)
    n_outs = len(out_avals)
    all_in_names = list(in_names) + list(out_names)
    if partition_name is not None:
        all_in_names.append(partition_name)
    donate = tuple(range(n_params, n_params + n_outs))

    def _body_fn(*args):
        operands = list(args)
        if partition_name is not None:
            operands.append(bass2jax.partition_id_tensor())
        outs = bass2jax._bass_exec_p.bind(
            *operands,
            out_avals=tuple(out_avals),
            in_names=tuple(all_in_names),
            out_names=tuple(out_names),
            lowering_input_output_aliases=(),
            sim_require_finite=True,
            sim_require_nnan=True,
            nc=nc,
        )
        return tuple(outs)

    devices = jax.devices()[:n_cores]
    if n_cores == 1:
        fn = jax.jit(_body_fn, donate_argnums=donate, keep_unused=True)
    else:
        mesh = Mesh(np.asarray(devices), ("core",))
        fn = jax.jit(
            shard_map(_body_fn, mesh=mesh,
                      in_specs=(PartitionSpec("core"),) * (n_params + n_outs),
                      out_specs=(PartitionSpec("core"),) * n_outs,
                      check_rep=False),
            donate_argnums=donate, keep_unused=True,
        )

    def run(per_core_inputs):
        if n_cores == 1:
            ins = [np.asarray(per_core_inputs[0][n]) for n in in_names]
            zeros = [np.zeros(s, d) for s, d in zero_shapes]
        else:
            ins = [np.concatenate([np.asarray(per_core_inputs[c][n])
                                   for c in range(n_cores)], axis=0) for n in in_names]
            zeros = [np.zeros((n_cores * s[0], *s[1:]), d) for s, d in zero_shapes]
        out_arrs = fn(*ins, *zeros)
        if n_cores == 1:
            return [{name: np.asarray(out_arrs[i]) for i, name in enumerate(out_names)}]
        res = []
        for c in range(n_cores):
            d = {}
            for i, name in enumerate(out_names):
                full = np.asarray(out_arrs[i])
                d[name] = full.reshape(n_cores, *out_avals[i].shape)[c]
            res.append(d)
        return res

    run.fn = fn
    run.in_names = in_names
    run.out_names = out_names
    run.zero_shapes = zero_shapes
    return run


_RUNNER = None


def _get_runner():
    global _RUNNER
    if _RUNNER is None:
        nc = _build_program(T)
        _RUNNER = _make_runner(nc, NCORES)
    return _RUNNER


def kernel(**inputs):
    import time as _time
    global _RUNNER
    p = {k: np.asarray(v) for k, v in inputs.items()}
    consts = _host_prep(p)
    x = p['x'].astype(np.float32)            # [B, T, D]
    per_core = []
    for b in range(B):
        m = dict(consts)
        m['xt'] = np.ascontiguousarray(x[b].T)
        per_core.append(m)
    res = None
    for attempt in range(3):
        try:
            run = _get_runner()
            res = run(per_core)
            break
        except Exception:
            # transient NRT exec faults have been observed on the first
            # execution of a freshly compiled NEFF; rebuild the jitted
            # callable (NEFF comes from the compile cache) and retry.
            _RUNNER = None
            if attempt == 2:
                raise
            _time.sleep(2.0)
    out = np.stack([res[b]['outT'].T for b in range(B)], axis=0)
    return np.ascontiguousarray(out, dtype=np.float32)
